# revision 43
# baseline (speedup 1.0000x reference)
"""Trainium2 Bass kernel for late-interaction retrieval scoring (FLUKE+).

Math per doc n (see reference):
  sims[q,t] = q_emb[q] . doc[n,t]                       (late interaction)
  pts[q]    = soft-top3 aggregation of sims[q,:]        (softmax(top3/T).top3)
  base      = sum_q wq[q] pts[q]
  ASC: pmax = max_t sims; stats (mean/max/std/frac) -> MLP -> calib;
       asc = (sum_q wq pmax) * (1+tanh(...)); total = blend*base+(1-blend)*asc
  MGS: for k=1..3, k-gram mean-pooled renormalized doc embs, MaxSim:
       sims_k[q,t] = (sum_i sims[q,t+i]) / ||sum_i doc[t+i]||
       total += gw[k] * sum_q wq max_t sims_k
  TIR: total += relu(pts*qm @ w1 + b1) @ w2 + b2

Engine split per supergroup of 32 docs (v5 — balanced across 5 engines):
  - DMA streams docT [D=128, t] bf16 plus e2T = (d_t + d_{t+1})/sqrt(2) in
    fp8e4m3 (host-added; feeds only the Square).
  - norm2sq[t] = ||d_t+d_{t+1}||^2 via ACT Square(e2) + PE ones-selector
    reduce (selector value 2.0 undoes the host 1/sqrt(2) and doubles the g2
    products in the same single matmul per doc); g2[t] = d_t.d_{t+2} from
    one DVE bf16-2x multiply; norm3sq = nr2[t]+nr2[t+1]+2*g2[t]-1 combined
    on GPSIMD; inv = exp(-0.5*ln(.)) on ACT, bf16.
  - inv is spilled to a DRAM scratch and re-read with partition-replicating
    DMAs (one per 32-row strip, all 8 groups at once) so the per-(q,doc)
    inverse-norm tile lands in SBUF as bf16 with no PE broadcast matmul.
  - per group of 4 docs: sims via col-tiled matmuls (M=32, tile_position);
    top-3 via DVE max8 on the bf16 SBUF copy of sims.
  - num2/num3 (k-gram numerators) = shifted adds of sims on GPSIMD into a
    [2, 179] packed tile with a -1e9 poison pad; k-gram scores = one DVE
    bf16-2x multiply + one two-region max-reduce.
  - softmax(mgs_logits) and sigmoid(asc_blend) are folded into host-prepped
    matmul weights (wqx) and combine constants (cc12), shortening the
    finishing combine; finishing per-strip math is vectorized over all rows.
  - emission is software-pipelined: supergroup sg+1's prep chain (DMA, pq
    products, gram matmuls, inv, spill+broadcast) is emitted inside sg's
    group loop under tc.high_priority so it overlaps.
"""

import os
import numpy as np
import ml_dtypes

# ---- problem constants (hardcoded; kernel.py must be self-contained)
N, NQ, ND, D = 4096, 32, 180, 128
TOPK, TEMP, MAXK = 3, 0.1, 3
NCORES = 8
NDOC = N // NCORES            # 512 docs per core
GROUP = 4                     # docs packed per col-tiled psum tile
SUPER = 32                    # docs per supergroup
NW2, NW3 = ND - 1, ND - 2     # 179, 178
NW = NW2 + NW3                # 357
RW = 2 * NW2                  # 358: padded [2, 179] k-gram region width
BF = ml_dtypes.bfloat16
F8 = ml_dtypes.float8_e4m3fn

_CACHE = {}


def _build_program(ndoc):
    """Build the SPMD Bass program for one core processing `ndoc` docs."""
    import concourse.bass as bass
    import concourse.tile as tile
    from concourse import mybir
    from contextlib import ExitStack

    f32, bf16 = mybir.dt.float32, mybir.dt.bfloat16
    f8 = mybir.dt.float8e4
    AF = mybir.ActivationFunctionType
    OP = mybir.AluOpType

    NSG = ndoc // SUPER       # supergroups
    NG = ndoc // GROUP        # total groups (= finishing tile width)
    GPS = SUPER // GROUP      # groups per supergroup = 8
    DBG_SKIP = set(os.environ.get("DBG_SKIP", "").split(","))

    nc = bass.Bass()

    # ---------------- DRAM I/O ----------------
    docT = nc.dram_tensor("docT", [128, ndoc * ND], bf16, kind="ExternalInput")
    e2T_d = nc.dram_tensor("e2T", [128, ndoc * ND], f8, kind="ExternalInput")
    qT_d = nc.dram_tensor("qT", [128, NQ], bf16, kind="ExternalInput")
    selg_d = nc.dram_tensor("selg", [128, 32 * GPS], bf16, kind="ExternalInput")
    wq32_d = nc.dram_tensor("wq32", [128, 32], f32, kind="ExternalInput")
    qmn32_d = nc.dram_tensor("qmn32", [128, 32], f32, kind="ExternalInput")
    tirw1_d = nc.dram_tensor("tirw1", [128, 64], f32, kind="ExternalInput")
    tirw2_d = nc.dram_tensor("tirw2", [128, 32], f32, kind="ExternalInput")
    tirb1_d = nc.dram_tensor("tirb1", [128, 1], f32, kind="ExternalInput")
    ascw1_d = nc.dram_tensor("ascw1", [128, 96], f32, kind="ExternalInput")
    ascw2_d = nc.dram_tensor("ascw2", [128, 32], f32, kind="ExternalInput")
    ascb1_d = nc.dram_tensor("ascb1", [128, 1], f32, kind="ExternalInput")
    ascb2x2_d = nc.dram_tensor("ascb2x2", [128, 1], f32, kind="ExternalInput")
    tirb2_d = nc.dram_tensor("tirb2", [128, 1], f32, kind="ExternalInput")
    qm4_d = nc.dram_tensor("qm4", [128, 1], f32, kind="ExternalInput")
    wqx_d = nc.dram_tensor("wqx", [128, 96], f32, kind="ExternalInput")
    cc12_d = nc.dram_tensor("cc12", [128, 2], f32, kind="ExternalInput")
    ident_d = nc.dram_tensor("ident", [128, 128], f32, kind="ExternalInput")
    out_d = nc.dram_tensor("out", [GROUP, NG], f32, kind="ExternalOutput")
    invd = nc.dram_tensor("invd", [128, (ndoc // SUPER) * RW], bf16, kind="Internal")
    # [4 strips, 32 rows, sg, t] view for the per-strip broadcast reads
    invd_v = invd.rearrange("(r s) (g t) -> r s g t", s=32, t=RW)

    docT_v = docT.rearrange("p (d t) -> p d t", t=ND)
    e2T_v = e2T_d.rearrange("p (d t) -> p d t", t=ND)

    with ExitStack() as ctx:
        tc = ctx.enter_context(tile.TileContext(nc))
        const = ctx.enter_context(tc.tile_pool(name="const", bufs=1))
        dpool = ctx.enter_context(tc.tile_pool(name="dpool", bufs=3))
        ppool = ctx.enter_context(tc.tile_pool(name="ppool", bufs=2))
        gpool = ctx.enter_context(tc.tile_pool(name="gpool", bufs=3))
        ipool = ctx.enter_context(tc.tile_pool(name="ipool", bufs=3))
        work = ctx.enter_context(tc.tile_pool(name="work", bufs=6))
        coll = ctx.enter_context(tc.tile_pool(name="coll", bufs=1))
        fin = ctx.enter_context(tc.tile_pool(name="fin", bufs=1))
        mainps = ctx.enter_context(ExitStack())
        ps = mainps.enter_context(tc.tile_pool(name="ps", bufs=5, space="PSUM"))
        psg = mainps.enter_context(tc.tile_pool(name="psg", bufs=2, space="PSUM"))

        # ---------------- constants (DMA order tuned) ----------------
        # dt(0) is emitted before any const DMA so the first supergroup's
        # products start ASAP; hot consts (selg/qT/esel) follow, cold consts
        # trail the second doc tile.
        qT = const.tile([128, NQ], bf16)
        selg = const.tile([128, 32 * GPS], bf16)
        wq32 = const.tile([128, 32], f32)
        qmn32 = const.tile([128, 32], f32)
        tirw1 = const.tile([128, 64], f32)
        tirw2 = const.tile([128, 32], f32)
        tirb1 = const.tile([128, 1], f32)
        ascw1 = const.tile([128, 96], f32)
        ascw2 = const.tile([128, 32], f32)
        ascb1 = const.tile([128, 1], f32)
        ascb2x2 = const.tile([128, 1], f32)
        tirb2 = const.tile([128, 1], f32)
        qm4 = const.tile([128, 1], f32)
        wqx = const.tile([128, 96], f32)
        cc12 = const.tile([128, 2], f32)
        ident = const.tile([128, 128], f32)

        def emit_hot_consts():
            nc.sync.dma_start(out=selg, in_=selg_d[:])
            nc.sync.dma_start(out=qT, in_=qT_d[:])

        def emit_cold_consts():
            nc.sync.dma_start(out=wq32, in_=wq32_d[:])
            nc.sync.dma_start(out=qmn32, in_=qmn32_d[:])
            nc.sync.dma_start(out=tirw1, in_=tirw1_d[:])
            nc.sync.dma_start(out=tirw2, in_=tirw2_d[:])
            nc.sync.dma_start(out=tirb1, in_=tirb1_d[:])
            nc.sync.dma_start(out=ascw1, in_=ascw1_d[:])
            nc.sync.dma_start(out=ascw2, in_=ascw2_d[:])
            nc.sync.dma_start(out=ascb1, in_=ascb1_d[:])
            nc.sync.dma_start(out=ascb2x2, in_=ascb2x2_d[:])
            nc.sync.dma_start(out=tirb2, in_=tirb2_d[:])
            nc.sync.dma_start(out=qm4, in_=qm4_d[:])
            nc.sync.dma_start(out=wqx, in_=wqx_d[:])
            nc.sync.dma_start(out=cc12, in_=cc12_d[:])
            nc.sync.dma_start(out=ident, in_=ident_d[:])

        b_zero = const.tile([128, 1], f32)
        nc.vector.memset(b_zero, 0.0)
        b_eps = const.tile([128, 1], f32)
        nc.vector.memset(b_eps, 1e-6)
        b_m1eps = const.tile([128, 1], f32)
        nc.vector.memset(b_m1eps, -1.0 + 1e-6)

        # ---------------- collectors ----------------
        top8c = coll.tile([128, NG, 8], f32)
        red23c = coll.tile([128, NG, 2], f32)

        # ---------------- main loop (software-pipelined emission) ----------
        # Engines execute their queues in emission order, so supergroup
        # sg+1's prep chain (DMA -> pq products -> gram matmuls -> inv) is
        # emitted interleaved INTO supergroup sg's group loop. Otherwise the
        # chain serializes behind the whole j-loop on every engine queue.
        tiles = {}
        PRIO = int(os.environ.get("PRIO", "80"))

        def emit_dma(sg):
            if sg >= NSG:
                return
            d0 = sg * SUPER
            dt_ = dpool.tile([128, SUPER, ND], bf16, tag="dt")
            e2t = dpool.tile([128, SUPER, ND], f8, tag="e2")
            nc.sync.dma_start(out=dt_[:, 0:16, :], in_=docT_v[:, d0:d0 + 16, :])
            nc.scalar.dma_start(out=e2t[:, 0:16, :], in_=e2T_v[:, d0:d0 + 16, :])
            nc.sync.dma_start(out=dt_[:, 16:32, :], in_=docT_v[:, d0 + 16:d0 + SUPER, :])
            nc.scalar.dma_start(out=e2t[:, 16:32, :], in_=e2T_v[:, d0 + 16:d0 + SUPER, :])
            tiles[sg] = {"dt": dt_, "e2": e2t}

        def emit_pq(sg, half, _prio=None):
            with tc.high_priority(offset=PRIO):
                return _emit_pq_inner(sg, half)

        def _emit_pq_inner(sg, half):
            # shared product tile pq: cols [0:178] = d_t.d_{t+2} (DVE mult,
            # bf16 2x — needs 4B-aligned offsets; the x2 folds into the
            # GPSIMD norm3 combine), cols [178:357] = (d_t+d_{t+1})^2 (ACT)
            t = tiles[sg]
            if half == 0:
                pq_t = ppool.tile([128, SUPER, NW], bf16, tag="pq")
                t["pq"] = pq_t
            if "gram" in DBG_SKIP:
                if half == 0:
                    nc.vector.memset(t["pq"], 1.0)
                return
            pq, dt_, e2 = t["pq"], t["dt"], t["e2"]
            dd = slice(16 * half, 16 * (half + 1))
            nc.vector.tensor_mul(pq[:, dd, 0:NW3], dt_[:, dd, 0:NW3],
                                 dt_[:, dd, 2:ND])
            for q in (0, 1):
                dq = slice(16 * half + 8 * q, 16 * half + 8 * (q + 1))
                nc.scalar.activation(out=pq[:, dq, NW3:NW],
                                     in_=e2[:, dq, 0:NW2],
                                     func=AF.Square, bias=b_zero, scale=1.0)

        def emit_gram(sg, jlist):
            with tc.high_priority(offset=PRIO):
                return _emit_gram_inner(sg, jlist)

        def _emit_gram_inner(sg, jlist):
            # gram reduction over D via ones-selector matmuls (ONE matmul
            # per doc streaming both regions — interleaved psum accumulation
            # groups on a tile_position corrupt each other).
            # gsum row 32b+j (doc 4j+b): [0:178] = g2, [178:357] = norm2sq
            t = tiles[sg]
            if 0 in jlist:
                gsum_new = psg.tile([128, 512], f32, tag="gsum")
                t["gsum_t"] = gsum_new
            if "gram" in DBG_SKIP:
                return
            gsum = t["gsum_t"][:, 0:NW]
            for j in jlist:
                sel = selg[:, 32 * j:32 * (j + 1)]
                for b in range(GROUP):
                    d = GROUP * j + b
                    nc.tensor.matmul(gsum[32 * b:32 * (b + 1), :], sel,
                                     t["pq"][:, d, :],
                                     start=(j == 0), stop=(j == GPS - 1),
                                     tile_position=(0, 32 * b), skip_group_check=True)

        def emit_inv(sg):
            with tc.high_priority(offset=PRIO):
                return _emit_inv_inner(sg)

        def _emit_inv_inner(sg):
            # inv2 = rsqrt(norm2sq); norm3sq = nr2[t]+nr2[t+1]+2*g2[t]-1
            # (the gram selector carries a 2.0 so the gsum g2-region is
            # already doubled and the sq-region is the exact norm2sq).
            # inv layout [2, 179]: [0:179] = inv2, [179:357] = inv3, [357] pad
            # (pad = exp(0) = 1 against the -1e9-poisoned numerator pad).
            # inv is spilled to DRAM so the per-group q-row broadcast can be
            # a single partition-replicating DMA instead of a PE matmul.
            t = tiles[sg]
            inv = gpool.tile([128, RW], bf16, tag="inv")
            if "gram" in DBG_SKIP:
                nc.vector.memset(inv, 1.0)
                t["inv"] = inv
                return
            gsum = t["gsum_t"][:, 0:NW]
            g_sb = gpool.tile([128, NW], bf16, tag="gsb")
            nc.scalar.copy(g_sb, gsum)
            nr2 = g_sb[:, NW3:NW]                      # [128, 179]
            n3a = gpool.tile([128, NW3], f32, tag="n3a")
            nc.gpsimd.tensor_add(n3a, nr2[:, 0:NW3], nr2[:, 1:NW2])
            n3b = gpool.tile([128, NW3], f32, tag="n3b")
            nc.gpsimd.tensor_add(n3b, n3a, g_sb[:, 0:NW3])
            lnn = gpool.tile([128, RW], f32, tag="lnn")
            nc.gpsimd.memset(lnn[:, NW:RW], 0.0)
            nc.scalar.activation(out=lnn[:, 0:NW2], in_=nr2,
                                 func=AF.Ln, bias=b_eps, scale=1.0)
            nc.scalar.activation(out=lnn[:, NW2:NW2 + NW3], in_=n3b,
                                 func=AF.Ln, bias=b_m1eps, scale=1.0)
            nc.scalar.activation(out=inv, in_=lnn, func=AF.Exp,
                                 bias=b_zero, scale=-0.5)
            t["inv"] = inv

        def emit_spill(sg):
            with tc.high_priority(offset=PRIO):
                return _emit_spill_inner(sg)

        def _emit_spill_inner(sg):
            # spill inv to DRAM, then broadcast rows {32b+j} to all 32
            # q-rows of each strip via partition-replicating DMAs (one DMA
            # per strip covers all 8 groups; bf16 SBUF operands keep the
            # k-gram multiply in the DVE 2x mode). All on the SP queue so a
            # blocked DMA head never starves a compute engine.
            t = tiles[sg]
            nc.sync.dma_start(out=invd[:, RW * sg:RW * (sg + 1)], in_=t["inv"])
            iball = ipool.tile([128, GPS, RW], bf16, tag="iball")
            t["iball"] = iball
            for r in range(GROUP):
                src = invd_v[r:r + 1, 0:8, sg:sg + 1, :].rearrange(
                    "r s g t -> g r s t").broadcast_to([32, 1, 8, RW])
                nc.sync.dma_start(out=iball[32 * r:32 * (r + 1), :, :], in_=src)

        def emit_groupA(sg, j):
            # sims matmuls, top-8, numerator adds. Returns the deferred
            # multiply+reduce closure args (run 2 groups later so the DVE
            # queue never blocks on the inv broadcast round-trip).
            t = tiles[sg]
            dt_ = t["dt"]
            gg = GPS * sg + j
            invb_sb = t["iball"][:, j, :]
            ps_sims_t = ps.tile([128, 512], f32, tag="sims")
            ps_sims = ps_sims_t[:, 0:ND]
            for b in range(GROUP):
                d = GROUP * j + b
                rows = slice(32 * b, 32 * (b + 1))
                nc.tensor.matmul(ps_sims[rows, :], qT, dt_[:, d, :],
                                 start=True, stop=True,
                                 tile_position=(0, 32 * b), skip_group_check=True)

            # sims -> SBUF (ACT, bf16) for max8 + numerator adds
            s_sims = work.tile([128, ND], bf16, tag="ssims")
            nc.scalar.copy(s_sims, ps_sims)
            if "max8" in DBG_SKIP:
                nc.vector.tensor_copy(top8c[:, gg, :], s_sims[:, 0:8])
            else:
                nc.vector.max(top8c[:, gg, :], s_sims)

            if "kmax" in DBG_SKIP:
                nc.vector.memset(red23c[:, gg, :], 0.0)
                return None
            # num2/num3 shifted adds on GPSIMD (all-SBUF), packed [2, 179]
            # with a -1e9 poison pad so one reduce covers both k's
            s_num23 = work.tile([128, RW], bf16, tag="snum23")
            nc.gpsimd.memset(s_num23[:, RW - 1:RW], -1e9)
            nc.gpsimd.tensor_add(s_num23[:, 0:NW2], s_sims[:, 0:NW2],
                                 s_sims[:, 1:ND])
            nc.gpsimd.tensor_add(s_num23[:, NW2:NW2 + NW3],
                                 s_num23[:, 0:NW3], s_sims[:, 2:ND])
            return (s_num23, invb_sb, gg)

        def emit_groupB(args):
            if args is None:
                return
            s_num23, invb_sb, gg = args
            # k-gram scores: bf16 2x multiply, then one max-reduce for both k
            scr23 = work.tile([128, RW], bf16, tag="scr23")
            nc.vector.tensor_mul(scr23, s_num23, invb_sb)
            nc.vector.tensor_reduce(
                out=red23c[:, gg, :],
                in_=scr23.rearrange("p (k t) -> p k t", k=2),
                op=OP.max, axis=mybir.AxisListType.X)

        # prologue: chain for sg 0 (+ DMA for sg 1)
        emit_dma(0)
        emit_hot_consts()
        emit_dma(1)
        emit_cold_consts()
        emit_pq(0, 0)
        emit_pq(0, 1)
        emit_gram(0, [0, 1, 2, 3])
        emit_gram(0, [4, 5, 6, 7])
        emit_inv(0)
        emit_spill(0)
        from collections import deque
        pending = deque()
        for sg in range(NSG):
            nxt = sg + 1
            for j in range(GPS):
                pending.append(emit_groupA(sg, j))
                if len(pending) > 2:
                    emit_groupB(pending.popleft())
                if nxt < NSG:
                    if j == 0:
                        emit_dma(nxt + 1)
                        emit_pq(nxt, 0)
                    elif j == 1:
                        emit_pq(nxt, 1)
                    elif j in (4, 5):
                        emit_gram(nxt, [4 * (j - 4) + k for k in range(4)])
                    elif j == 6:
                        emit_inv(nxt)
                    elif j == 7:
                        emit_spill(nxt)
            tiles.pop(sg, None)
        while pending:
            emit_groupB(pending.popleft())

        # ---------------- finishing phase ----------------
        if "fin" in DBG_SKIP:
            zz = fin.tile([128, NG], f32)
            nc.vector.memset(zz, 0.0)
            for b in range(GROUP):
                nc.sync.dma_start(out=out_d[b:b + 1, :], in_=zz[32 * b:32 * b + 1, :])
        mainps.close()
        ps = ctx.enter_context(tc.tile_pool(name="psf", bufs=1, space="PSUM"))
        if "fin" in DBG_SKIP:
            return nc

        def _dbg_stop():
            zz = fin.tile([128, NG], f32, tag="dbgz")
            nc.vector.memset(zz, 0.0)
            for b in range(GROUP):
                nc.sync.dma_start(out=out_d[b:b + 1, :], in_=zz[32 * b:32 * b + 1, :])
            return nc
        topv = top8c[:, :, 0:TOPK]              # [128, NG, 3]
        pmax = top8c[:, :, 0:1]                 # [128, NG, 1]

        # pts = softmax(topv/T).topv
        exps = fin.tile([128, NG, TOPK], f32)
        nc.scalar.activation(out=exps, in_=topv, func=AF.Exp,
                             bias=b_zero, scale=1.0 / TEMP)
        den = fin.tile([128, NG], f32)
        nc.vector.reduce_sum(den, exps, axis=mybir.AxisListType.X)
        wnum = fin.tile([128, NG, TOPK], f32)
        nc.vector.tensor_mul(wnum, exps, topv)
        pnum = fin.tile([128, NG], f32)
        nc.vector.reduce_sum(pnum, wnum, axis=mybir.AxisListType.X)
        rden = fin.tile([128, NG], f32)
        nc.vector.reciprocal(rden, den)
        pts = fin.tile([128, NG], f32)
        nc.vector.tensor_mul(pts, pnum, rden)
        # padded = pts * qm (qm broadcast per partition block)
        pts_t = fin.tile([128, NG], f32)
        nc.vector.tensor_scalar(out=pts_t, in0=pts, scalar1=qm4, scalar2=None,
                                op0=OP.mult)
        pmax2 = fin.tile([128, NG], f32)
        nc.vector.tensor_mul(pmax2, pmax[:, :, 0], pmax[:, :, 0])
        if "fin1" in DBG_SKIP:
            return _dbg_stop()

        # stats matmuls, col+row tiled per strip: all outputs on rows {32b}
        ps_stat_t = ps.tile([128, 512], f32, tag="stat")
        ps_stat = ps_stat_t[:, 0:3 * NG]        # asc_base | mean | msq
        ps_stat2_t = ps.tile([128, 512], f32, tag="stat2")
        ps_stat2 = ps_stat2_t[:, 0:3 * NG]      # red2w | red3w | base
        ps_mlp_t = ps.tile([128, 512], f32, tag="mlp")
        ps_mlp = ps_mlp_t[:, 0:NG]              # H
        ps_base = ps_stat2[:, 2 * NG:3 * NG]
        pmax_2d = top8c[:, :, 0]                # [128, NG] strided view
        for b in range(GROUP):
            rows = slice(32 * b, 32 * (b + 1))
            tp = (32 * b, 32 * b)
            nc.tensor.matmul(ps_stat[rows, 0:NG], wq32[rows, :],
                             pmax_2d[rows, :], start=True, stop=True,
                             tile_position=tp, skip_group_check=True)
            nc.tensor.matmul(ps_stat[rows, NG:2 * NG], qmn32[rows, :],
                             pmax_2d[rows, :], start=True, stop=True,
                             tile_position=tp, skip_group_check=True)
            nc.tensor.matmul(ps_stat[rows, 2 * NG:3 * NG], qmn32[rows, :],
                             pmax2[rows, :], start=True, stop=True,
                             tile_position=tp, skip_group_check=True)
            nc.tensor.matmul(ps_base[rows, :], wqx[rows, 0:32],
                             pts[rows, :], start=True, stop=True,
                             tile_position=tp, skip_group_check=True)
            nc.tensor.matmul(ps_stat2[rows, 0:NG], wqx[rows, 32:64],
                             red23c[rows, :, 0], start=True, stop=True,
                             tile_position=tp, skip_group_check=True)
            nc.tensor.matmul(ps_stat2[rows, NG:2 * NG], wqx[rows, 64:96],
                             red23c[rows, :, 1], start=True, stop=True,
                             tile_position=tp, skip_group_check=True)

        # single ordered whole-bank reads (PSUM banks must not be read while
        # PE still writes other columns of the same bank)
        stat_sb = fin.tile([128, 3 * NG], f32)
        nc.scalar.copy(stat_sb, ps_stat)
        stat2_sb = fin.tile([128, 3 * NG], f32)
        nc.scalar.copy(stat2_sb, ps_stat2)
        if "fin2" in DBG_SKIP:
            return _dbg_stop()
        # mx: max over q (transpose + free-dim reduce + transpose back)
        pmax_sb = fin.tile([128, NG], f32)
        nc.vector.tensor_copy(pmax_sb, pmax_2d)
        ps_pT_t = ps.tile([128, 512], f32, tag="ptrans")
        ps_pT = ps_pT_t[:, 0:128]
        nc.tensor.transpose(ps_pT[0:NG, :], pmax_sb, ident)
        mxT = fin.tile([128, GROUP], f32)
        nc.vector.reduce_max(mxT[0:NG, :], ps_pT[0:NG, :].rearrange(
            "g (b q) -> g b q", q=32), axis=mybir.AxisListType.X)
        mxpad = fin.tile([128, 128], f32)
        nc.vector.memset(mxpad, 0.0)
        nc.vector.tensor_copy(
            mxpad[0:NG, :].rearrange("g (b o) -> g b o", o=32)[:, :, 0:1],
            mxT[0:NG, :].rearrange("g (b o) -> g b o", o=1))
        ps_mxb_t = ps.tile([128, 512], f32, tag="ptrans")
        ps_mxb = ps_mxb_t[:, 0:128]
        nc.tensor.transpose(ps_mxb[:, 0:NG], mxpad[0:NG, :], ident[0:NG, 0:NG])

        if "fin3" in DBG_SKIP:
            return _dbg_stop()
        # ---- per-strip finishing, vectorized over all 128 rows (unused
        # rows hold finite zeros; only rows {32b} are read downstream) ----
        F = fin.tile([128, 3 * NG], f32)    # cols: mean | mx | std, rows {32b}
        scrA = fin.tile([128, NG], f32)
        scrB = fin.tile([128, NG], f32)
        nc.vector.tensor_copy(F[:, 0:NG], stat_sb[:, NG:2 * NG])
        nc.scalar.copy(F[:, NG:2 * NG], ps_mxb[:, 0:NG])
        # var = msq - mean^2 -> std = exp(0.5*ln(var + 1e-6))
        nc.vector.tensor_mul(scrA, F[:, 0:NG], F[:, 0:NG])
        nc.vector.tensor_sub(scrB, stat_sb[:, 2 * NG:3 * NG], scrA)
        nc.scalar.activation(out=scrA, in_=scrB, func=AF.Ln,
                             bias=b_eps, scale=1.0)
        nc.scalar.activation(out=F[:, 2 * NG:3 * NG], in_=scrA,
                             func=AF.Exp, bias=b_zero, scale=0.5)

        # ASC MLP via K=1 accumulation over the 3 features
        for b in range(GROUP):
            row = slice(32 * b, 32 * b + 1)
            for s in range(3):
                nc.tensor.matmul(ps_mlp[32 * b:32 * (b + 1), :],
                                 ascw1[row, 32 * s:32 * (s + 1)],
                                 F[row, s * NG:(s + 1) * NG],
                                 start=(s == 0), stop=(s == 2),
                                 tile_position=(32 * b, 32 * b),
                                 skip_group_check=True)
        Hs = fin.tile([128, NG], f32)
        nc.scalar.activation(out=Hs, in_=ps_mlp, func=AF.Relu, bias=ascb1, scale=1.0)
        ps_calsc_t = ps.tile([128, 512], f32, tag="cal")
        ps_cal = ps_calsc_t[:, 0:NG]
        for b in range(GROUP):
            nc.tensor.matmul(ps_cal[32 * b:32 * (b + 1), :],
                             ascw2[32 * b:32 * (b + 1), :],
                             Hs[32 * b:32 * (b + 1), :], start=True, stop=True,
                             tile_position=(32 * b, 32 * b), skip_group_check=True)
        cal_sb = fin.tile([128, NG], f32)
        nc.scalar.copy(cal_sb, ps_cal)

        if "fin4" in DBG_SKIP:
            return _dbg_stop()
        # TIR MLP (row-tiled K=32); A/B in separate banks (PE-W vs ACT-R hazard)
        ps_tirA_t = ps.tile([128, 512], f32, tag="tirA")
        ps_tirA = ps_tirA_t[:, 0:NG]
        ps_tirB_t = ps.tile([128, 512], f32, tag="tirB")
        ps_tirB = ps_tirB_t[:, 0:NG]
        tir_dst = [(ps_tirA, 0), (ps_tirA, 64), (ps_tirB, 0), (ps_tirB, 64)]
        for b in range(GROUP):
            dst, o = tir_dst[b]
            nc.tensor.matmul(dst[o:o + 64, :], tirw1[32 * b:32 * (b + 1), :],
                             pts_t[32 * b:32 * (b + 1), :], start=True, stop=True,
                             tile_position=(32 * b, o), skip_group_check=True)
        HsA = fin.tile([128, NG], f32)
        HsB = fin.tile([128, NG], f32)
        nc.scalar.activation(out=HsA, in_=ps_tirA, func=AF.Relu, bias=tirb1, scale=1.0)
        nc.scalar.activation(out=HsB, in_=ps_tirB, func=AF.Relu, bias=tirb1, scale=1.0)
        ps_tv_t = ps.tile([128, 512], f32, tag="tv")
        ps_tv = ps_tv_t[:, 0:NG]
        tir_src = [(HsA, 0), (HsA, 64), (HsB, 0), (HsB, 64)]
        for b in range(GROUP):
            src_t, o = tir_src[b]
            nc.tensor.matmul(ps_tv[32 * b:32 * (b + 1), :], tirw2[o:o + 64, :],
                             src_t[o:o + 64, :], start=True, stop=True,
                             tile_position=(o, 32 * b), skip_group_check=True)

        tv_sb = fin.tile([128, NG], f32)
        nc.scalar.copy(tv_sb, ps_tv)
        if "fin6" in DBG_SKIP:
            return _dbg_stop()
        # combine per strip on row {32b}:
        # total = blend*base + (1-blend)*asc_base*(1+calib)
        #         + gw0*asc_base + gw1*red2w + gw2*red3w + tirv + tir_b2
        # vectorized over all rows; the gw/blend scalars are folded into
        # the host-prepped matmul weights (wqx) and cc12 so the chain is:
        # total = asc_base * (C1 - C2/(exp(2x+2b2)+1)) + blend*base
        #         + gw1*red2w + gw2*red3w + tirv + tir_b2
        # with C1 = 2(1-blend)+gw0, C2 = 2(1-blend)
        tot = fin.tile([128, NG], f32)
        asc_base = stat_sb[:, 0:NG]
        nc.scalar.activation(out=scrA, in_=cal_sb,
                             func=AF.Exp, bias=ascb2x2, scale=2.0)
        nc.vector.tensor_scalar(out=scrA, in0=scrA,
                                scalar1=1.0, scalar2=None, op0=OP.add)
        nc.vector.reciprocal(scrB, scrA)
        nc.vector.tensor_scalar(out=scrB, in0=scrB,
                                scalar1=cc12[:, 1:2], scalar2=cc12[:, 0:1],
                                op0=OP.mult, op1=OP.add)
        nc.vector.tensor_mul(scrB, scrB, asc_base)
        nc.vector.tensor_add(tot, scrB, stat2_sb[:, 2 * NG:3 * NG])
        nc.vector.tensor_add(tot, tot, stat2_sb[:, 0:NG])
        nc.vector.tensor_add(tot, tot, stat2_sb[:, NG:2 * NG])
        nc.vector.tensor_add(tot, tot, tv_sb)
        nc.vector.tensor_scalar(out=tot, in0=tot,
                                scalar1=tirb2, scalar2=None, op0=OP.add)
        for b in range(GROUP):
            nc.sync.dma_start(out=out_d[b:b + 1, :], in_=tot[32 * b:32 * b + 1, :])

    return nc


def _legalize_single_wait(nc):
    """Walrus (this compile path) accepts at most one sync wait per
    instruction; offload extra waits onto preceding EventSemaphore
    instructions on the same engine queue."""
    from concourse import mybir
    ctr = [0]
    for bb in nc.main_func.blocks:
        il = bb.instructions
        out = []
        for inst in il:
            si = inst.sync_info
            if si is not None and len(si.on_wait) > 1:
                waits = list(si.on_wait)
                eng = nc.engines[inst.engine]
                for w in waits[:-1]:
                    ev = eng._isa(
                        nc.isa.Opcode.NEURON_ISA_TPB_OPCODE_NOP, {})
                    ev.sync_info = mybir.SyncInfo(on_wait=[w], on_update=[])
                    ctr[0] += 1
                    try:
                        nc.register_instruction(ev)
                    except Exception:
                        pass
                    out.append(ev)
                inst.sync_info = mybir.SyncInfo(on_wait=[waits[-1]],
                                                on_update=list(si.on_update))
            out.append(inst)
        bb.instructions = out
    return nc


def _host_prep(inputs, ndoc_per_core, ncores):
    """Shard + lay out inputs for the SPMD program. Returns list of in_maps."""
    q = np.asarray(inputs["query_embs"], np.float32)          # [NQ, D]
    docs = np.asarray(inputs["doc_embs"], np.float32)         # [N, ND, D]
    w = np.asarray(inputs["importance_weights"], np.float32)  # [NQ]
    qm = np.asarray(inputs["query_mask"]).astype(np.float32)  # [NQ]
    asc_w1 = np.asarray(inputs["asc_w1"], np.float32)
    asc_b1 = np.asarray(inputs["asc_b1"], np.float32)
    asc_w2 = np.asarray(inputs["asc_w2"], np.float32)
    asc_b2 = np.float32(inputs["asc_b2"])
    asc_blend = np.float32(inputs["asc_blend"])
    mgs_logits = np.asarray(inputs["mgs_logits"], np.float32)
    tir_w1 = np.asarray(inputs["tir_w1"], np.float32)
    tir_b1 = np.asarray(inputs["tir_b1"], np.float32)
    tir_w2 = np.asarray(inputs["tir_w2"], np.float32)
    tir_b2 = np.float32(inputs["tir_b2"])

    wq = (w * qm).astype(np.float32)
    nvalid = float(qm.sum())
    frac = nvalid / NQ

    # frac folded into ASC layer-1 bias; feats order = [mean, mx, std]
    b1p = asc_b1 + frac * asc_w1[3, :]
    w1p = asc_w1[:3, :]                                       # [3, 32]

    GPS = SUPER // GROUP
    qT = np.ascontiguousarray(q.T).astype(BF)                 # [128, 32]
    # each doc's gram row replicated to rows {j, j+8, j+16, j+24} of its
    # strip so every partition holds a valid (positive) norm — keeps the
    # Ln in the inv path NaN-free on otherwise-unused rows.
    # selector value 2.0: doubles the g2 products and undoes the host-side
    # 1/sqrt(2) scaling of e2 (whose squares then sum to exactly norm2sq)
    selg = np.zeros((128, 32 * GPS), BF)
    for j in range(GPS):
        for kk in range(4):
            selg[:, 32 * j + j + 8 * kk] = 2
    wq32 = np.zeros((128, 32), np.float32)
    wq32[:, 0] = np.tile(wq, 4)
    qmn32 = np.zeros((128, 32), np.float32)
    qmn32[:, 0] = np.tile(qm / max(nvalid, 1e-9), 4)
    tirw1 = np.tile(tir_w1, (4, 1)).astype(np.float32)        # [128, 64]
    tirw2 = np.zeros((128, 32), np.float32)
    tirw2[:, 0] = np.tile(tir_w2, 2)
    tirb1 = np.tile(tir_b1, 2).reshape(128, 1).astype(np.float32)
    ascw1 = np.zeros((128, 96), np.float32)
    for b in range(4):
        for s in range(3):
            ascw1[32 * b, 32 * s:32 * (s + 1)] = w1p[s, :]
    ascw2 = np.zeros((128, 32), np.float32)
    ascw2[:, 0] = np.tile(asc_w2, 4)
    ascb1 = np.tile(b1p, 4).reshape(128, 1).astype(np.float32)
    ascb2x2 = np.full((128, 1), 2.0 * asc_b2, np.float32)
    tirb2 = np.full((128, 1), tir_b2, np.float32)
    qm4 = np.tile(qm, 4).reshape(128, 1).astype(np.float32)
    gl = mgs_logits.astype(np.float64)
    gw = np.exp(gl - gl.max()); gw = (gw / gw.sum()).astype(np.float32)
    blend = np.float32(1.0 / (1.0 + np.exp(-float(asc_blend))))
    wqx = np.zeros((128, 96), np.float32)
    wqx[:, 0] = np.tile(wq * blend, 4)
    wqx[:, 32] = np.tile(wq * gw[1], 4)
    wqx[:, 64] = np.tile(wq * gw[2], 4)
    cc12 = np.zeros((128, 2), np.float32)
    cc12[:, 0] = 2.0 * (1.0 - blend) + gw[0]      # C1
    cc12[:, 1] = -2.0 * (1.0 - blend)             # -C2
    ident = np.eye(128, dtype=np.float32)

    shared = dict(qT=qT, selg=selg, wq32=wq32, qmn32=qmn32,
                  tirw1=tirw1, tirw2=tirw2, tirb1=tirb1, ascw1=ascw1,
                  ascw2=ascw2, ascb1=ascb1, ascb2x2=ascb2x2, tirb2=tirb2,
                  qm4=qm4, wqx=wqx, cc12=cc12, ident=ident)

    in_maps = []
    for c in range(ncores):
        sl = docs[c * ndoc_per_core:(c + 1) * ndoc_per_core]   # [ndoc, ND, D]
        dT = np.ascontiguousarray(sl.transpose(2, 0, 1)).astype(BF)  # [D, ndoc, ND]
        e2 = np.zeros_like(sl)
        e2[:, :ND - 1, :] = (sl[:, :ND - 1, :] + sl[:, 1:, :]) * np.float32(
            1.0 / np.sqrt(2.0))
        e2T = np.ascontiguousarray(e2.transpose(2, 0, 1)).astype(F8)
        m = dict(shared)
        m["docT"] = dT.reshape(128, ndoc_per_core * ND)
        m["e2T"] = e2T.reshape(128, ndoc_per_core * ND)
        in_maps.append(m)
    return in_maps


def _numpy_fallback(inputs):
    """Full-precision numpy implementation (only for non-all-ones masks)."""
    q = np.asarray(inputs["query_embs"], np.float64)
    docs = np.asarray(inputs["doc_embs"], np.float64)
    w = np.asarray(inputs["importance_weights"], np.float64)
    qm_b = np.asarray(inputs["query_mask"]).astype(bool)
    dm_b = np.asarray(inputs["doc_mask"]).astype(bool)
    NEG = -1e9
    qm = qm_b.astype(np.float64)
    wq = w * qm
    sims = np.einsum("qd,ntd->nqt", q, docs)
    sims = np.where(dm_b[:, None, :], sims, NEG)
    topv = -np.sort(-sims, axis=-1)[:, :, :TOPK]
    e = np.exp((topv - topv.max(-1, keepdims=True)) / TEMP)
    soft = e / e.sum(-1, keepdims=True)
    pts = (soft * topv).sum(-1)
    base = pts @ wq
    pmax = sims.max(-1)
    asc_base = pmax @ wq
    nvalid = qm.sum()
    mean = (pmax * qm).sum(-1) / nvalid
    mx = np.where(qm_b, pmax, NEG).max(-1)
    std = np.sqrt((((pmax - mean[:, None]) ** 2) * qm).sum(-1) / nvalid + 1e-6)
    frac = np.full_like(mean, nvalid / NQ)
    feats = np.stack([mean, mx, std, frac], -1)
    h = np.maximum(feats @ np.asarray(inputs["asc_w1"], np.float64)
                   + np.asarray(inputs["asc_b1"], np.float64), 0)
    calib = np.tanh(h @ np.asarray(inputs["asc_w2"], np.float64)
                    + float(inputs["asc_b2"]))
    asc_score = asc_base * (1.0 + calib)
    blend = 1 / (1 + np.exp(-float(inputs["asc_blend"])))
    total = blend * base + (1 - blend) * asc_score
    gl = np.asarray(inputs["mgs_logits"], np.float64)
    gw = np.exp(gl - gl.max()); gw /= gw.sum()
    dmf = dm_b.astype(np.float64)
    for k in range(1, MAXK + 1):
        if k == 1:
            dk, mk = docs, dm_b
        else:
            nw = ND - k + 1
            s = sum(docs[:, i:i + nw] for i in range(k)) / k
            dk = s / np.sqrt((s * s).sum(-1, keepdims=True) + 1e-12)
            mkf = dmf[:, 0:nw].copy()
            for i in range(1, k):
                mkf = mkf * dmf[:, i:i + nw]
            mk = mkf > 0.5
        sk = np.einsum("qd,nwd->nqw", q, dk)
        sk = np.where(mk[:, None, :], sk, NEG)
        total = total + gw[k - 1] * (sk.max(-1) @ wq)
    padded = pts * qm
    hres = np.maximum(padded @ np.asarray(inputs["tir_w1"], np.float64)
                      + np.asarray(inputs["tir_b1"], np.float64), 0)
    total = total + hres @ np.asarray(inputs["tir_w2"], np.float64) + float(inputs["tir_b2"])
    return total.astype(np.float32)


def kernel(**inputs):
    qm = np.asarray(inputs["query_mask"]).astype(bool)
    dm = np.asarray(inputs["doc_mask"]).astype(bool)
    if not (qm.all() and dm.all()):
        return _numpy_fallback(inputs)

    from concourse.bass_utils import run_bass_kernel_spmd

    key = ("prog", NDOC)
    if key not in _CACHE:
        _CACHE[key] = _legalize_single_wait(_build_program(NDOC))
    nc = _CACHE[key]

    in_maps = _host_prep(inputs, NDOC, NCORES)
    res = run_bass_kernel_spmd(nc, in_maps, list(range(NCORES)))
    out = np.empty((N,), np.float32)
    for c in range(NCORES):
        grid = res.results[c]["out"]            # [4, NG] -> doc = 4*g + b
        out[c * NDOC:(c + 1) * NDOC] = grid.T.reshape(-1)
    return out


if __name__ == "__main__":
    # quick CoreSim correctness check on a reduced doc count
    from concourse.bass_interp import CoreSim

    nd = int(os.environ.get("SIM_NDOC", "32"))
    rng = np.random.default_rng(0)

    def l2n(x):
        return x / np.sqrt((x * x).sum(-1, keepdims=True) + 1e-12)

    inputs = {
        "query_embs": l2n(rng.standard_normal((NQ, D))).astype(np.float32),
        "doc_embs": l2n(rng.standard_normal((nd, ND, D))).astype(np.float32),
        "importance_weights": rng.random(NQ).astype(np.float32),
        "query_mask": np.ones(NQ, bool),
        "doc_mask": np.ones((nd, ND), bool),
        "asc_w1": (rng.standard_normal((4, 32)) * 0.1).astype(np.float32),
        "asc_b1": np.zeros(32, np.float32),
        "asc_w2": (rng.standard_normal(32) * 0.1).astype(np.float32),
        "asc_b2": np.float32(0.0),
        "asc_blend": np.float32(0.5),
        "mgs_logits": (rng.standard_normal(3) * 0.1).astype(np.float32),
        "tir_w1": (rng.standard_normal((NQ, 64)) * 0.1).astype(np.float32),
        "tir_b1": np.zeros(64, np.float32),
        "tir_w2": (rng.standard_normal(64) * 0.1).astype(np.float32),
        "tir_b2": np.float32(0.0),
    }
    globals()["N"] = nd  # shrink problem for sim
    expected = _numpy_fallback(inputs)

    nc = _build_program(nd)
    in_maps = _host_prep(inputs, nd, 1)
    sim = CoreSim(nc)
    for k, v in in_maps[0].items():
        sim.tensor(k)[:] = v
    sim.simulate()
    grid = np.array(sim.tensor("out"))
    got = grid.T.reshape(-1)
    err = np.abs(got - expected)
    rel = err.max() / np.abs(expected).max()
    print("expected[:8]:", expected[:8])
    print("got[:8]     :", got[:8])
    print("max abs err:", err.max(), " rel:", rel)


# revision 48
# speedup vs baseline: 1.0047x; 1.0047x over previous
"""Trainium2 Bass kernel for late-interaction retrieval scoring (FLUKE+).

Math per doc n (see reference):
  sims[q,t] = q_emb[q] . doc[n,t]                       (late interaction)
  pts[q]    = soft-top3 aggregation of sims[q,:]        (softmax(top3/T).top3)
  base      = sum_q wq[q] pts[q]
  ASC: pmax = max_t sims; stats (mean/max/std/frac) -> MLP -> calib;
       asc = (sum_q wq pmax) * (1+tanh(...)); total = blend*base+(1-blend)*asc
  MGS: for k=1..3, k-gram mean-pooled renormalized doc embs, MaxSim:
       sims_k[q,t] = (sum_i sims[q,t+i]) / ||sum_i doc[t+i]||
       total += gw[k] * sum_q wq max_t sims_k
  TIR: total += relu(pts*qm @ w1 + b1) @ w2 + b2

Engine split per supergroup of 32 docs (v5 — balanced across 5 engines):
  - DMA streams docT [D=128, t] bf16 plus e2T = (d_t + d_{t+1})/sqrt(2) in
    fp8e4m3 (host-added; feeds only the Square).
  - norm2sq[t] = ||d_t+d_{t+1}||^2 via ACT Square(e2) + PE ones-selector
    reduce (selector value 2.0 undoes the host 1/sqrt(2) and doubles the g2
    products in the same single matmul per doc); g2[t] = d_t.d_{t+2} from
    one DVE bf16-2x multiply; norm3sq = nr2[t]+nr2[t+1]+2*g2[t]-1 combined
    on GPSIMD; inv = exp(-0.5*ln(.)) on ACT, bf16.
  - inv is spilled to a DRAM scratch and re-read with partition-replicating
    DMAs (one per 32-row strip, all 8 groups at once) so the per-(q,doc)
    inverse-norm tile lands in SBUF as bf16 with no PE broadcast matmul.
  - per group of 4 docs: sims via col-tiled matmuls (M=32, tile_position);
    top-3 via DVE max8 on the bf16 SBUF copy of sims.
  - num2/num3 (k-gram numerators) = shifted adds of sims on GPSIMD into a
    [2, 179] packed tile with a -1e9 poison pad; k-gram scores = one DVE
    bf16-2x multiply + one two-region max-reduce.
  - softmax(mgs_logits) and sigmoid(asc_blend) are folded into host-prepped
    matmul weights (wqx) and combine constants (cc12), shortening the
    finishing combine; finishing per-strip math is vectorized over all rows.
  - emission is software-pipelined: supergroup sg+1's prep chain (DMA, pq
    products, gram matmuls, inv, spill+broadcast) is emitted inside sg's
    group loop under tc.high_priority so it overlaps.
"""

import os
import numpy as np
import ml_dtypes

# ---- problem constants (hardcoded; kernel.py must be self-contained)
N, NQ, ND, D = 4096, 32, 180, 128
TOPK, TEMP, MAXK = 3, 0.1, 3
NCORES = 8
NDOC = N // NCORES            # 512 docs per core
GROUP = 4                     # docs packed per col-tiled psum tile
SUPER = 32                    # docs per supergroup
NW2, NW3 = ND - 1, ND - 2     # 179, 178
NW = NW2 + NW3                # 357
RW = 2 * NW2                  # 358: padded [2, 179] k-gram region width
BF = ml_dtypes.bfloat16
F8 = ml_dtypes.float8_e4m3fn

_CACHE = {}


def _build_program(ndoc):
    """Build the SPMD Bass program for one core processing `ndoc` docs."""
    import concourse.bass as bass
    import concourse.tile as tile
    from concourse import mybir
    from contextlib import ExitStack

    f32, bf16 = mybir.dt.float32, mybir.dt.bfloat16
    f8 = mybir.dt.float8e4
    AF = mybir.ActivationFunctionType
    OP = mybir.AluOpType

    NSG = ndoc // SUPER       # supergroups
    NG = ndoc // GROUP        # total groups (= finishing tile width)
    GPS = SUPER // GROUP      # groups per supergroup = 8
    DBG_SKIP = set(os.environ.get("DBG_SKIP", "").split(","))

    nc = bass.Bass()

    # ---------------- DRAM I/O ----------------
    docT = nc.dram_tensor("docT", [128, ndoc * ND], bf16, kind="ExternalInput")
    e2T_d = nc.dram_tensor("e2T", [128, ndoc * ND], f8, kind="ExternalInput")
    qT_d = nc.dram_tensor("qT", [128, NQ], bf16, kind="ExternalInput")
    selg_d = nc.dram_tensor("selg", [128, 32 * GPS], bf16, kind="ExternalInput")
    wq32_d = nc.dram_tensor("wq32", [128, 32], f32, kind="ExternalInput")
    qmn32_d = nc.dram_tensor("qmn32", [128, 32], f32, kind="ExternalInput")
    tirw1_d = nc.dram_tensor("tirw1", [128, 64], f32, kind="ExternalInput")
    tirw2_d = nc.dram_tensor("tirw2", [128, 32], f32, kind="ExternalInput")
    tirb1_d = nc.dram_tensor("tirb1", [128, 1], f32, kind="ExternalInput")
    ascw1_d = nc.dram_tensor("ascw1", [128, 96], f32, kind="ExternalInput")
    ascw2_d = nc.dram_tensor("ascw2", [128, 32], f32, kind="ExternalInput")
    ascb1_d = nc.dram_tensor("ascb1", [128, 1], f32, kind="ExternalInput")
    ascb2x2_d = nc.dram_tensor("ascb2x2", [128, 1], f32, kind="ExternalInput")
    tirb2_d = nc.dram_tensor("tirb2", [128, 1], f32, kind="ExternalInput")
    qm4_d = nc.dram_tensor("qm4", [128, 1], f32, kind="ExternalInput")
    wqx_d = nc.dram_tensor("wqx", [128, 96], f32, kind="ExternalInput")
    cc12_d = nc.dram_tensor("cc12", [128, 2], f32, kind="ExternalInput")
    ident_d = nc.dram_tensor("ident", [128, 128], f32, kind="ExternalInput")
    out_d = nc.dram_tensor("out", [GROUP, NG], f32, kind="ExternalOutput")
    invd = nc.dram_tensor("invd", [128, (ndoc // SUPER) * RW], bf16, kind="Internal")
    # [4 strips, 32 rows, sg, t] view for the per-strip broadcast reads
    invd_v = invd.rearrange("(r s) (g t) -> r s g t", s=32, t=RW)

    docT_v = docT.rearrange("p (d t) -> p d t", t=ND)
    e2T_v = e2T_d.rearrange("p (d t) -> p d t", t=ND)

    with ExitStack() as ctx:
        tc = ctx.enter_context(tile.TileContext(nc))
        const = ctx.enter_context(tc.tile_pool(name="const", bufs=1))
        dpool = ctx.enter_context(tc.tile_pool(name="dpool", bufs=3))
        ppool = ctx.enter_context(tc.tile_pool(name="ppool", bufs=2))
        gpool = ctx.enter_context(tc.tile_pool(name="gpool", bufs=3))
        ipool = ctx.enter_context(tc.tile_pool(name="ipool", bufs=3))
        work = ctx.enter_context(tc.tile_pool(name="work", bufs=6))
        coll = ctx.enter_context(tc.tile_pool(name="coll", bufs=1))
        fin = ctx.enter_context(tc.tile_pool(name="fin", bufs=1))
        mainps = ctx.enter_context(ExitStack())
        ps = mainps.enter_context(tc.tile_pool(name="ps", bufs=5, space="PSUM"))
        psg = mainps.enter_context(tc.tile_pool(name="psg", bufs=2, space="PSUM"))

        # ---------------- constants (DMA order tuned) ----------------
        # dt(0) is emitted before any const DMA so the first supergroup's
        # products start ASAP; hot consts (selg/qT/esel) follow, cold consts
        # trail the second doc tile.
        qT = const.tile([128, NQ], bf16)
        selg = const.tile([128, 32 * GPS], bf16)
        wq32 = const.tile([128, 32], f32)
        qmn32 = const.tile([128, 32], f32)
        tirw1 = const.tile([128, 64], f32)
        tirw2 = const.tile([128, 32], f32)
        tirb1 = const.tile([128, 1], f32)
        ascw1 = const.tile([128, 96], f32)
        ascw2 = const.tile([128, 32], f32)
        ascb1 = const.tile([128, 1], f32)
        ascb2x2 = const.tile([128, 1], f32)
        tirb2 = const.tile([128, 1], f32)
        qm4 = const.tile([128, 1], f32)
        wqx = const.tile([128, 96], f32)
        cc12 = const.tile([128, 2], f32)
        ident = const.tile([128, 128], f32)

        def emit_hot_consts():
            nc.sync.dma_start(out=selg, in_=selg_d[:])
            nc.sync.dma_start(out=qT, in_=qT_d[:])

        def emit_cold_consts():
            nc.sync.dma_start(out=wq32, in_=wq32_d[:])
            nc.sync.dma_start(out=qmn32, in_=qmn32_d[:])
            nc.sync.dma_start(out=tirw1, in_=tirw1_d[:])
            nc.sync.dma_start(out=tirw2, in_=tirw2_d[:])
            nc.sync.dma_start(out=tirb1, in_=tirb1_d[:])
            nc.sync.dma_start(out=ascw1, in_=ascw1_d[:])
            nc.sync.dma_start(out=ascw2, in_=ascw2_d[:])
            nc.sync.dma_start(out=ascb1, in_=ascb1_d[:])
            nc.sync.dma_start(out=ascb2x2, in_=ascb2x2_d[:])
            nc.sync.dma_start(out=tirb2, in_=tirb2_d[:])
            nc.sync.dma_start(out=qm4, in_=qm4_d[:])
            nc.sync.dma_start(out=wqx, in_=wqx_d[:])
            nc.sync.dma_start(out=cc12, in_=cc12_d[:])
            nc.sync.dma_start(out=ident, in_=ident_d[:])

        b_zero = const.tile([128, 1], f32)
        nc.vector.memset(b_zero, 0.0)
        b_eps = const.tile([128, 1], f32)
        nc.vector.memset(b_eps, 1e-6)
        b_m1eps = const.tile([128, 1], f32)
        nc.vector.memset(b_m1eps, -1.0 + 1e-6)

        # ---------------- collectors ----------------
        top8c = coll.tile([128, NG, 8], f32)
        red23c = coll.tile([128, NG, 2], f32)

        # ---------------- main loop (software-pipelined emission) ----------
        # Engines execute their queues in emission order, so supergroup
        # sg+1's prep chain (DMA -> pq products -> gram matmuls -> inv) is
        # emitted interleaved INTO supergroup sg's group loop. Otherwise the
        # chain serializes behind the whole j-loop on every engine queue.
        tiles = {}
        PRIO = int(os.environ.get("PRIO", "80"))
        PQPRIO = int(os.environ.get("PQPRIO", "40"))

        def emit_dma(sg):
            if sg >= NSG:
                return
            d0 = sg * SUPER
            dt_ = dpool.tile([128, SUPER, ND], bf16, tag="dt")
            e2t = dpool.tile([128, SUPER, ND], f8, tag="e2")
            nc.sync.dma_start(out=dt_[:, 0:16, :], in_=docT_v[:, d0:d0 + 16, :])
            nc.scalar.dma_start(out=e2t[:, 0:16, :], in_=e2T_v[:, d0:d0 + 16, :])
            nc.sync.dma_start(out=dt_[:, 16:32, :], in_=docT_v[:, d0 + 16:d0 + SUPER, :])
            nc.scalar.dma_start(out=e2t[:, 16:32, :], in_=e2T_v[:, d0 + 16:d0 + SUPER, :])
            tiles[sg] = {"dt": dt_, "e2": e2t}

        def emit_pq(sg, half, _prio=None):
            with tc.high_priority(offset=PQPRIO):
                return _emit_pq_inner(sg, half)

        def _emit_pq_inner(sg, half):
            # shared product tile pq: cols [0:178] = d_t.d_{t+2} (DVE mult,
            # bf16 2x — needs 4B-aligned offsets; the x2 folds into the
            # GPSIMD norm3 combine), cols [178:357] = (d_t+d_{t+1})^2 (ACT)
            t = tiles[sg]
            if half == 0:
                pq_t = ppool.tile([128, SUPER, NW], bf16, tag="pq")
                t["pq"] = pq_t
            if "gram" in DBG_SKIP:
                if half == 0:
                    nc.vector.memset(t["pq"], 1.0)
                return
            pq, dt_, e2 = t["pq"], t["dt"], t["e2"]
            dd = slice(16 * half, 16 * (half + 1))
            nc.vector.tensor_mul(pq[:, dd, 0:NW3], dt_[:, dd, 0:NW3],
                                 dt_[:, dd, 2:ND])
            for q in (0, 1):
                dq = slice(16 * half + 8 * q, 16 * half + 8 * (q + 1))
                nc.scalar.activation(out=pq[:, dq, NW3:NW],
                                     in_=e2[:, dq, 0:NW2],
                                     func=AF.Square, bias=b_zero, scale=1.0)

        def emit_gram(sg, jlist):
            with tc.high_priority(offset=PRIO):
                return _emit_gram_inner(sg, jlist)

        def _emit_gram_inner(sg, jlist):
            # gram reduction over D via ones-selector matmuls (ONE matmul
            # per doc streaming both regions — interleaved psum accumulation
            # groups on a tile_position corrupt each other).
            # gsum row 32b+j (doc 4j+b): [0:178] = g2, [178:357] = norm2sq
            t = tiles[sg]
            if 0 in jlist:
                gsum_new = psg.tile([128, 512], f32, tag="gsum")
                t["gsum_t"] = gsum_new
            if "gram" in DBG_SKIP:
                return
            gsum = t["gsum_t"][:, 0:NW]
            for j in jlist:
                sel = selg[:, 32 * j:32 * (j + 1)]
                for b in range(GROUP):
                    d = GROUP * j + b
                    nc.tensor.matmul(gsum[32 * b:32 * (b + 1), :], sel,
                                     t["pq"][:, d, :],
                                     start=(j == 0), stop=(j == GPS - 1),
                                     tile_position=(0, 32 * b), skip_group_check=True)

        def emit_inv(sg):
            with tc.high_priority(offset=PRIO):
                return _emit_inv_inner(sg)

        def _emit_inv_inner(sg):
            # inv2 = rsqrt(norm2sq); norm3sq = nr2[t]+nr2[t+1]+2*g2[t]-1
            # (the gram selector carries a 2.0 so the gsum g2-region is
            # already doubled and the sq-region is the exact norm2sq).
            # inv layout [2, 179]: [0:179] = inv2, [179:357] = inv3, [357] pad
            # (pad = exp(0) = 1 against the -1e9-poisoned numerator pad).
            # inv is spilled to DRAM so the per-group q-row broadcast can be
            # a single partition-replicating DMA instead of a PE matmul.
            t = tiles[sg]
            inv = gpool.tile([128, RW], bf16, tag="inv")
            if "gram" in DBG_SKIP:
                nc.vector.memset(inv, 1.0)
                t["inv"] = inv
                return
            gsum = t["gsum_t"][:, 0:NW]
            g_sb = gpool.tile([128, NW], bf16, tag="gsb")
            nc.scalar.copy(g_sb, gsum)
            nr2 = g_sb[:, NW3:NW]                      # [128, 179]
            n3a = gpool.tile([128, NW3], f32, tag="n3a")
            nc.gpsimd.tensor_add(n3a, nr2[:, 0:NW3], nr2[:, 1:NW2])
            n3b = gpool.tile([128, NW3], f32, tag="n3b")
            nc.gpsimd.tensor_add(n3b, n3a, g_sb[:, 0:NW3])
            lnn = gpool.tile([128, RW], f32, tag="lnn")
            nc.gpsimd.memset(lnn[:, NW:RW], 0.0)
            nc.scalar.activation(out=lnn[:, 0:NW2], in_=nr2,
                                 func=AF.Ln, bias=b_eps, scale=1.0)
            nc.scalar.activation(out=lnn[:, NW2:NW2 + NW3], in_=n3b,
                                 func=AF.Ln, bias=b_m1eps, scale=1.0)
            nc.scalar.activation(out=inv, in_=lnn, func=AF.Exp,
                                 bias=b_zero, scale=-0.5)
            t["inv"] = inv

        def emit_spill(sg):
            with tc.high_priority(offset=PRIO):
                return _emit_spill_inner(sg)

        def _emit_spill_inner(sg):
            # spill inv to DRAM, then broadcast rows {32b+j} to all 32
            # q-rows of each strip via partition-replicating DMAs (one DMA
            # per strip covers all 8 groups; bf16 SBUF operands keep the
            # k-gram multiply in the DVE 2x mode). All on the SP queue so a
            # blocked DMA head never starves a compute engine.
            t = tiles[sg]
            nc.sync.dma_start(out=invd[:, RW * sg:RW * (sg + 1)], in_=t["inv"])
            iball = ipool.tile([128, GPS, RW], bf16, tag="iball")
            t["iball"] = iball
            for r in range(GROUP):
                src = invd_v[r:r + 1, 0:8, sg:sg + 1, :].rearrange(
                    "r s g t -> g r s t").broadcast_to([32, 1, 8, RW])
                nc.sync.dma_start(out=iball[32 * r:32 * (r + 1), :, :], in_=src)

        def emit_groupA(sg, j):
            # sims matmuls, top-8, numerator adds. Returns the deferred
            # multiply+reduce closure args (run 2 groups later so the DVE
            # queue never blocks on the inv broadcast round-trip).
            t = tiles[sg]
            dt_ = t["dt"]
            gg = GPS * sg + j
            invb_sb = t["iball"][:, j, :]
            ps_sims_t = ps.tile([128, 512], f32, tag="sims")
            ps_sims = ps_sims_t[:, 0:ND]
            for b in range(GROUP):
                d = GROUP * j + b
                rows = slice(32 * b, 32 * (b + 1))
                nc.tensor.matmul(ps_sims[rows, :], qT, dt_[:, d, :],
                                 start=True, stop=True,
                                 tile_position=(0, 32 * b), skip_group_check=True)

            # sims -> SBUF (ACT, bf16) for max8 + numerator adds
            s_sims = work.tile([128, ND], bf16, tag="ssims")
            nc.scalar.copy(s_sims, ps_sims)
            if "max8" in DBG_SKIP:
                nc.vector.tensor_copy(top8c[:, gg, :], s_sims[:, 0:8])
            else:
                nc.vector.max(top8c[:, gg, :], s_sims)

            if "kmax" in DBG_SKIP:
                nc.vector.memset(red23c[:, gg, :], 0.0)
                return None
            # num2/num3 shifted adds on GPSIMD (all-SBUF), packed [2, 179]
            # with a -1e9 poison pad so one reduce covers both k's
            s_num23 = work.tile([128, RW], bf16, tag="snum23")
            nc.gpsimd.memset(s_num23[:, RW - 1:RW], -1e9)
            nc.gpsimd.tensor_add(s_num23[:, 0:NW2], s_sims[:, 0:NW2],
                                 s_sims[:, 1:ND])
            nc.gpsimd.tensor_add(s_num23[:, NW2:NW2 + NW3],
                                 s_num23[:, 0:NW3], s_sims[:, 2:ND])
            return (s_num23, invb_sb, gg)

        def emit_groupB(args):
            if args is None:
                return
            s_num23, invb_sb, gg = args
            # k-gram scores: bf16 2x multiply, then one max-reduce for both k
            scr23 = work.tile([128, RW], bf16, tag="scr23")
            nc.vector.tensor_mul(scr23, s_num23, invb_sb)
            nc.vector.tensor_reduce(
                out=red23c[:, gg, :],
                in_=scr23.rearrange("p (k t) -> p k t", k=2),
                op=OP.max, axis=mybir.AxisListType.X)

        # prologue: chain for sg 0 (+ DMA for sg 1)
        emit_dma(0)
        emit_hot_consts()
        emit_dma(1)
        emit_cold_consts()
        emit_pq(0, 0)
        emit_pq(0, 1)
        emit_gram(0, [0, 1, 2, 3])
        emit_gram(0, [4, 5, 6, 7])
        emit_inv(0)
        emit_spill(0)
        from collections import deque
        pending = deque()
        for sg in range(NSG):
            nxt = sg + 1
            for j in range(GPS):
                pending.append(emit_groupA(sg, j))
                if len(pending) > 2:
                    emit_groupB(pending.popleft())
                if nxt < NSG:
                    if j == 0:
                        emit_dma(nxt + 1)
                        emit_pq(nxt, 0)
                    elif j == 1:
                        emit_pq(nxt, 1)
                    elif j in (4, 5):
                        emit_gram(nxt, [4 * (j - 4) + k for k in range(4)])
                    elif j == 6:
                        emit_inv(nxt)
                    elif j == 7:
                        emit_spill(nxt)
            tiles.pop(sg, None)
        while pending:
            emit_groupB(pending.popleft())

        # ---------------- finishing phase ----------------
        if "fin" in DBG_SKIP:
            zz = fin.tile([128, NG], f32)
            nc.vector.memset(zz, 0.0)
            for b in range(GROUP):
                nc.sync.dma_start(out=out_d[b:b + 1, :], in_=zz[32 * b:32 * b + 1, :])
        mainps.close()
        ps = ctx.enter_context(tc.tile_pool(name="psf", bufs=1, space="PSUM"))
        if "fin" in DBG_SKIP:
            return nc

        def _dbg_stop():
            zz = fin.tile([128, NG], f32, tag="dbgz")
            nc.vector.memset(zz, 0.0)
            for b in range(GROUP):
                nc.sync.dma_start(out=out_d[b:b + 1, :], in_=zz[32 * b:32 * b + 1, :])
            return nc
        topv = top8c[:, :, 0:TOPK]              # [128, NG, 3]
        pmax = top8c[:, :, 0:1]                 # [128, NG, 1]

        # pts = softmax(topv/T).topv
        exps = fin.tile([128, NG, TOPK], f32)
        nc.scalar.activation(out=exps, in_=topv, func=AF.Exp,
                             bias=b_zero, scale=1.0 / TEMP)
        den = fin.tile([128, NG], f32)
        nc.vector.reduce_sum(den, exps, axis=mybir.AxisListType.X)
        wnum = fin.tile([128, NG, TOPK], f32)
        nc.vector.tensor_mul(wnum, exps, topv)
        pnum = fin.tile([128, NG], f32)
        nc.vector.reduce_sum(pnum, wnum, axis=mybir.AxisListType.X)
        rden = fin.tile([128, NG], f32)
        nc.vector.reciprocal(rden, den)
        pts = fin.tile([128, NG], f32)
        nc.vector.tensor_mul(pts, pnum, rden)
        # padded = pts * qm (qm broadcast per partition block)
        pts_t = fin.tile([128, NG], f32)
        nc.vector.tensor_scalar(out=pts_t, in0=pts, scalar1=qm4, scalar2=None,
                                op0=OP.mult)
        pmax2 = fin.tile([128, NG], f32)
        nc.vector.tensor_mul(pmax2, pmax[:, :, 0], pmax[:, :, 0])
        if "fin1" in DBG_SKIP:
            return _dbg_stop()

        # stats matmuls, col+row tiled per strip: all outputs on rows {32b}
        ps_stat_t = ps.tile([128, 512], f32, tag="stat")
        ps_stat = ps_stat_t[:, 0:3 * NG]        # asc_base | mean | msq
        ps_stat2_t = ps.tile([128, 512], f32, tag="stat2")
        ps_stat2 = ps_stat2_t[:, 0:3 * NG]      # red2w | red3w | base
        ps_mlp_t = ps.tile([128, 512], f32, tag="mlp")
        ps_mlp = ps_mlp_t[:, 0:NG]              # H
        ps_base = ps_stat2[:, 2 * NG:3 * NG]
        pmax_2d = top8c[:, :, 0]                # [128, NG] strided view
        for b in range(GROUP):
            rows = slice(32 * b, 32 * (b + 1))
            tp = (32 * b, 32 * b)
            nc.tensor.matmul(ps_stat[rows, 0:NG], wq32[rows, :],
                             pmax_2d[rows, :], start=True, stop=True,
                             tile_position=tp, skip_group_check=True)
            nc.tensor.matmul(ps_stat[rows, NG:2 * NG], qmn32[rows, :],
                             pmax_2d[rows, :], start=True, stop=True,
                             tile_position=tp, skip_group_check=True)
            nc.tensor.matmul(ps_stat[rows, 2 * NG:3 * NG], qmn32[rows, :],
                             pmax2[rows, :], start=True, stop=True,
                             tile_position=tp, skip_group_check=True)
            nc.tensor.matmul(ps_base[rows, :], wqx[rows, 0:32],
                             pts[rows, :], start=True, stop=True,
                             tile_position=tp, skip_group_check=True)
            nc.tensor.matmul(ps_stat2[rows, 0:NG], wqx[rows, 32:64],
                             red23c[rows, :, 0], start=True, stop=True,
                             tile_position=tp, skip_group_check=True)
            nc.tensor.matmul(ps_stat2[rows, NG:2 * NG], wqx[rows, 64:96],
                             red23c[rows, :, 1], start=True, stop=True,
                             tile_position=tp, skip_group_check=True)

        # single ordered whole-bank reads (PSUM banks must not be read while
        # PE still writes other columns of the same bank)
        stat_sb = fin.tile([128, 3 * NG], f32)
        nc.scalar.copy(stat_sb, ps_stat)
        stat2_sb = fin.tile([128, 3 * NG], f32)
        nc.scalar.copy(stat2_sb, ps_stat2)
        if "fin2" in DBG_SKIP:
            return _dbg_stop()
        # mx: max over q (transpose + free-dim reduce + transpose back)
        pmax_sb = fin.tile([128, NG], f32)
        nc.vector.tensor_copy(pmax_sb, pmax_2d)
        ps_pT_t = ps.tile([128, 512], f32, tag="ptrans")
        ps_pT = ps_pT_t[:, 0:128]
        nc.tensor.transpose(ps_pT[0:NG, :], pmax_sb, ident)
        mxT = fin.tile([128, GROUP], f32)
        nc.vector.reduce_max(mxT[0:NG, :], ps_pT[0:NG, :].rearrange(
            "g (b q) -> g b q", q=32), axis=mybir.AxisListType.X)
        mxpad = fin.tile([128, 128], f32)
        nc.vector.memset(mxpad, 0.0)
        nc.vector.tensor_copy(
            mxpad[0:NG, :].rearrange("g (b o) -> g b o", o=32)[:, :, 0:1],
            mxT[0:NG, :].rearrange("g (b o) -> g b o", o=1))
        ps_mxb_t = ps.tile([128, 512], f32, tag="ptrans")
        ps_mxb = ps_mxb_t[:, 0:128]
        nc.tensor.transpose(ps_mxb[:, 0:NG], mxpad[0:NG, :], ident[0:NG, 0:NG])

        if "fin3" in DBG_SKIP:
            return _dbg_stop()
        # ---- per-strip finishing, vectorized over all 128 rows (unused
        # rows hold finite zeros; only rows {32b} are read downstream) ----
        F = fin.tile([128, 3 * NG], f32)    # cols: mean | mx | std, rows {32b}
        scrA = fin.tile([128, NG], f32)
        scrB = fin.tile([128, NG], f32)
        nc.vector.tensor_copy(F[:, 0:NG], stat_sb[:, NG:2 * NG])
        nc.scalar.copy(F[:, NG:2 * NG], ps_mxb[:, 0:NG])
        # var = msq - mean^2 -> std = exp(0.5*ln(var + 1e-6))
        nc.vector.tensor_mul(scrA, F[:, 0:NG], F[:, 0:NG])
        nc.vector.tensor_sub(scrB, stat_sb[:, 2 * NG:3 * NG], scrA)
        nc.scalar.activation(out=scrA, in_=scrB, func=AF.Ln,
                             bias=b_eps, scale=1.0)
        nc.scalar.activation(out=F[:, 2 * NG:3 * NG], in_=scrA,
                             func=AF.Exp, bias=b_zero, scale=0.5)

        # ASC MLP via K=1 accumulation over the 3 features
        for b in range(GROUP):
            row = slice(32 * b, 32 * b + 1)
            for s in range(3):
                nc.tensor.matmul(ps_mlp[32 * b:32 * (b + 1), :],
                                 ascw1[row, 32 * s:32 * (s + 1)],
                                 F[row, s * NG:(s + 1) * NG],
                                 start=(s == 0), stop=(s == 2),
                                 tile_position=(32 * b, 32 * b),
                                 skip_group_check=True)
        Hs = fin.tile([128, NG], f32)
        nc.scalar.activation(out=Hs, in_=ps_mlp, func=AF.Relu, bias=ascb1, scale=1.0)
        ps_calsc_t = ps.tile([128, 512], f32, tag="cal")
        ps_cal = ps_calsc_t[:, 0:NG]
        for b in range(GROUP):
            nc.tensor.matmul(ps_cal[32 * b:32 * (b + 1), :],
                             ascw2[32 * b:32 * (b + 1), :],
                             Hs[32 * b:32 * (b + 1), :], start=True, stop=True,
                             tile_position=(32 * b, 32 * b), skip_group_check=True)
        cal_sb = fin.tile([128, NG], f32)
        nc.scalar.copy(cal_sb, ps_cal)

        if "fin4" in DBG_SKIP:
            return _dbg_stop()
        # TIR MLP (row-tiled K=32); A/B in separate banks (PE-W vs ACT-R hazard)
        ps_tirA_t = ps.tile([128, 512], f32, tag="tirA")
        ps_tirA = ps_tirA_t[:, 0:NG]
        ps_tirB_t = ps.tile([128, 512], f32, tag="tirB")
        ps_tirB = ps_tirB_t[:, 0:NG]
        tir_dst = [(ps_tirA, 0), (ps_tirA, 64), (ps_tirB, 0), (ps_tirB, 64)]
        for b in range(GROUP):
            dst, o = tir_dst[b]
            nc.tensor.matmul(dst[o:o + 64, :], tirw1[32 * b:32 * (b + 1), :],
                             pts_t[32 * b:32 * (b + 1), :], start=True, stop=True,
                             tile_position=(32 * b, o), skip_group_check=True)
        HsA = fin.tile([128, NG], f32)
        HsB = fin.tile([128, NG], f32)
        nc.scalar.activation(out=HsA, in_=ps_tirA, func=AF.Relu, bias=tirb1, scale=1.0)
        nc.scalar.activation(out=HsB, in_=ps_tirB, func=AF.Relu, bias=tirb1, scale=1.0)
        ps_tv_t = ps.tile([128, 512], f32, tag="tv")
        ps_tv = ps_tv_t[:, 0:NG]
        tir_src = [(HsA, 0), (HsA, 64), (HsB, 0), (HsB, 64)]
        for b in range(GROUP):
            src_t, o = tir_src[b]
            nc.tensor.matmul(ps_tv[32 * b:32 * (b + 1), :], tirw2[o:o + 64, :],
                             src_t[o:o + 64, :], start=True, stop=True,
                             tile_position=(o, 32 * b), skip_group_check=True)

        tv_sb = fin.tile([128, NG], f32)
        nc.scalar.copy(tv_sb, ps_tv)
        if "fin6" in DBG_SKIP:
            return _dbg_stop()
        # combine per strip on row {32b}:
        # total = blend*base + (1-blend)*asc_base*(1+calib)
        #         + gw0*asc_base + gw1*red2w + gw2*red3w + tirv + tir_b2
        # vectorized over all rows; the gw/blend scalars are folded into
        # the host-prepped matmul weights (wqx) and cc12 so the chain is:
        # total = asc_base * (C1 - C2/(exp(2x+2b2)+1)) + blend*base
        #         + gw1*red2w + gw2*red3w + tirv + tir_b2
        # with C1 = 2(1-blend)+gw0, C2 = 2(1-blend)
        tot = fin.tile([128, NG], f32)
        asc_base = stat_sb[:, 0:NG]
        nc.scalar.activation(out=scrA, in_=cal_sb,
                             func=AF.Exp, bias=ascb2x2, scale=2.0)
        nc.vector.tensor_scalar(out=scrA, in0=scrA,
                                scalar1=1.0, scalar2=None, op0=OP.add)
        nc.vector.reciprocal(scrB, scrA)
        nc.vector.tensor_scalar(out=scrB, in0=scrB,
                                scalar1=cc12[:, 1:2], scalar2=cc12[:, 0:1],
                                op0=OP.mult, op1=OP.add)
        nc.vector.tensor_mul(scrB, scrB, asc_base)
        nc.vector.tensor_add(tot, scrB, stat2_sb[:, 2 * NG:3 * NG])
        nc.vector.tensor_add(tot, tot, stat2_sb[:, 0:NG])
        nc.vector.tensor_add(tot, tot, stat2_sb[:, NG:2 * NG])
        nc.vector.tensor_add(tot, tot, tv_sb)
        nc.vector.tensor_scalar(out=tot, in0=tot,
                                scalar1=tirb2, scalar2=None, op0=OP.add)
        for b in range(GROUP):
            nc.sync.dma_start(out=out_d[b:b + 1, :], in_=tot[32 * b:32 * b + 1, :])

    return nc


def _legalize_single_wait(nc):
    """Walrus (this compile path) accepts at most one sync wait per
    instruction; offload extra waits onto preceding EventSemaphore
    instructions on the same engine queue."""
    from concourse import mybir
    ctr = [0]
    for bb in nc.main_func.blocks:
        il = bb.instructions
        out = []
        for inst in il:
            si = inst.sync_info
            if si is not None and len(si.on_wait) > 1:
                waits = list(si.on_wait)
                eng = nc.engines[inst.engine]
                for w in waits[:-1]:
                    ev = eng._isa(
                        nc.isa.Opcode.NEURON_ISA_TPB_OPCODE_NOP, {})
                    ev.sync_info = mybir.SyncInfo(on_wait=[w], on_update=[])
                    ctr[0] += 1
                    try:
                        nc.register_instruction(ev)
                    except Exception:
                        pass
                    out.append(ev)
                inst.sync_info = mybir.SyncInfo(on_wait=[waits[-1]],
                                                on_update=list(si.on_update))
            out.append(inst)
        bb.instructions = out
    return nc


def _host_prep(inputs, ndoc_per_core, ncores):
    """Shard + lay out inputs for the SPMD program. Returns list of in_maps."""
    q = np.asarray(inputs["query_embs"], np.float32)          # [NQ, D]
    docs = np.asarray(inputs["doc_embs"], np.float32)         # [N, ND, D]
    w = np.asarray(inputs["importance_weights"], np.float32)  # [NQ]
    qm = np.asarray(inputs["query_mask"]).astype(np.float32)  # [NQ]
    asc_w1 = np.asarray(inputs["asc_w1"], np.float32)
    asc_b1 = np.asarray(inputs["asc_b1"], np.float32)
    asc_w2 = np.asarray(inputs["asc_w2"], np.float32)
    asc_b2 = np.float32(inputs["asc_b2"])
    asc_blend = np.float32(inputs["asc_blend"])
    mgs_logits = np.asarray(inputs["mgs_logits"], np.float32)
    tir_w1 = np.asarray(inputs["tir_w1"], np.float32)
    tir_b1 = np.asarray(inputs["tir_b1"], np.float32)
    tir_w2 = np.asarray(inputs["tir_w2"], np.float32)
    tir_b2 = np.float32(inputs["tir_b2"])

    wq = (w * qm).astype(np.float32)
    nvalid = float(qm.sum())
    frac = nvalid / NQ

    # frac folded into ASC layer-1 bias; feats order = [mean, mx, std]
    b1p = asc_b1 + frac * asc_w1[3, :]
    w1p = asc_w1[:3, :]                                       # [3, 32]

    GPS = SUPER // GROUP
    qT = np.ascontiguousarray(q.T).astype(BF)                 # [128, 32]
    # each doc's gram row replicated to rows {j, j+8, j+16, j+24} of its
    # strip so every partition holds a valid (positive) norm — keeps the
    # Ln in the inv path NaN-free on otherwise-unused rows.
    # selector value 2.0: doubles the g2 products and undoes the host-side
    # 1/sqrt(2) scaling of e2 (whose squares then sum to exactly norm2sq)
    selg = np.zeros((128, 32 * GPS), BF)
    for j in range(GPS):
        for kk in range(4):
            selg[:, 32 * j + j + 8 * kk] = 2
    wq32 = np.zeros((128, 32), np.float32)
    wq32[:, 0] = np.tile(wq, 4)
    qmn32 = np.zeros((128, 32), np.float32)
    qmn32[:, 0] = np.tile(qm / max(nvalid, 1e-9), 4)
    tirw1 = np.tile(tir_w1, (4, 1)).astype(np.float32)        # [128, 64]
    tirw2 = np.zeros((128, 32), np.float32)
    tirw2[:, 0] = np.tile(tir_w2, 2)
    tirb1 = np.tile(tir_b1, 2).reshape(128, 1).astype(np.float32)
    ascw1 = np.zeros((128, 96), np.float32)
    for b in range(4):
        for s in range(3):
            ascw1[32 * b, 32 * s:32 * (s + 1)] = w1p[s, :]
    ascw2 = np.zeros((128, 32), np.float32)
    ascw2[:, 0] = np.tile(asc_w2, 4)
    ascb1 = np.tile(b1p, 4).reshape(128, 1).astype(np.float32)
    ascb2x2 = np.full((128, 1), 2.0 * asc_b2, np.float32)
    tirb2 = np.full((128, 1), tir_b2, np.float32)
    qm4 = np.tile(qm, 4).reshape(128, 1).astype(np.float32)
    gl = mgs_logits.astype(np.float64)
    gw = np.exp(gl - gl.max()); gw = (gw / gw.sum()).astype(np.float32)
    blend = np.float32(1.0 / (1.0 + np.exp(-float(asc_blend))))
    wqx = np.zeros((128, 96), np.float32)
    wqx[:, 0] = np.tile(wq * blend, 4)
    wqx[:, 32] = np.tile(wq * gw[1], 4)
    wqx[:, 64] = np.tile(wq * gw[2], 4)
    cc12 = np.zeros((128, 2), np.float32)
    cc12[:, 0] = 2.0 * (1.0 - blend) + gw[0]      # C1
    cc12[:, 1] = -2.0 * (1.0 - blend)             # -C2
    ident = np.eye(128, dtype=np.float32)

    shared = dict(qT=qT, selg=selg, wq32=wq32, qmn32=qmn32,
                  tirw1=tirw1, tirw2=tirw2, tirb1=tirb1, ascw1=ascw1,
                  ascw2=ascw2, ascb1=ascb1, ascb2x2=ascb2x2, tirb2=tirb2,
                  qm4=qm4, wqx=wqx, cc12=cc12, ident=ident)

    in_maps = []
    for c in range(ncores):
        sl = docs[c * ndoc_per_core:(c + 1) * ndoc_per_core]   # [ndoc, ND, D]
        dT = np.ascontiguousarray(sl.transpose(2, 0, 1)).astype(BF)  # [D, ndoc, ND]
        e2 = np.zeros_like(sl)
        e2[:, :ND - 1, :] = (sl[:, :ND - 1, :] + sl[:, 1:, :]) * np.float32(
            1.0 / np.sqrt(2.0))
        e2T = np.ascontiguousarray(e2.transpose(2, 0, 1)).astype(F8)
        m = dict(shared)
        m["docT"] = dT.reshape(128, ndoc_per_core * ND)
        m["e2T"] = e2T.reshape(128, ndoc_per_core * ND)
        in_maps.append(m)
    return in_maps


def _numpy_fallback(inputs):
    """Full-precision numpy implementation (only for non-all-ones masks)."""
    q = np.asarray(inputs["query_embs"], np.float64)
    docs = np.asarray(inputs["doc_embs"], np.float64)
    w = np.asarray(inputs["importance_weights"], np.float64)
    qm_b = np.asarray(inputs["query_mask"]).astype(bool)
    dm_b = np.asarray(inputs["doc_mask"]).astype(bool)
    NEG = -1e9
    qm = qm_b.astype(np.float64)
    wq = w * qm
    sims = np.einsum("qd,ntd->nqt", q, docs)
    sims = np.where(dm_b[:, None, :], sims, NEG)
    topv = -np.sort(-sims, axis=-1)[:, :, :TOPK]
    e = np.exp((topv - topv.max(-1, keepdims=True)) / TEMP)
    soft = e / e.sum(-1, keepdims=True)
    pts = (soft * topv).sum(-1)
    base = pts @ wq
    pmax = sims.max(-1)
    asc_base = pmax @ wq
    nvalid = qm.sum()
    mean = (pmax * qm).sum(-1) / nvalid
    mx = np.where(qm_b, pmax, NEG).max(-1)
    std = np.sqrt((((pmax - mean[:, None]) ** 2) * qm).sum(-1) / nvalid + 1e-6)
    frac = np.full_like(mean, nvalid / NQ)
    feats = np.stack([mean, mx, std, frac], -1)
    h = np.maximum(feats @ np.asarray(inputs["asc_w1"], np.float64)
                   + np.asarray(inputs["asc_b1"], np.float64), 0)
    calib = np.tanh(h @ np.asarray(inputs["asc_w2"], np.float64)
                    + float(inputs["asc_b2"]))
    asc_score = asc_base * (1.0 + calib)
    blend = 1 / (1 + np.exp(-float(inputs["asc_blend"])))
    total = blend * base + (1 - blend) * asc_score
    gl = np.asarray(inputs["mgs_logits"], np.float64)
    gw = np.exp(gl - gl.max()); gw /= gw.sum()
    dmf = dm_b.astype(np.float64)
    for k in range(1, MAXK + 1):
        if k == 1:
            dk, mk = docs, dm_b
        else:
            nw = ND - k + 1
            s = sum(docs[:, i:i + nw] for i in range(k)) / k
            dk = s / np.sqrt((s * s).sum(-1, keepdims=True) + 1e-12)
            mkf = dmf[:, 0:nw].copy()
            for i in range(1, k):
                mkf = mkf * dmf[:, i:i + nw]
            mk = mkf > 0.5
        sk = np.einsum("qd,nwd->nqw", q, dk)
        sk = np.where(mk[:, None, :], sk, NEG)
        total = total + gw[k - 1] * (sk.max(-1) @ wq)
    padded = pts * qm
    hres = np.maximum(padded @ np.asarray(inputs["tir_w1"], np.float64)
                      + np.asarray(inputs["tir_b1"], np.float64), 0)
    total = total + hres @ np.asarray(inputs["tir_w2"], np.float64) + float(inputs["tir_b2"])
    return total.astype(np.float32)


def kernel(**inputs):
    qm = np.asarray(inputs["query_mask"]).astype(bool)
    dm = np.asarray(inputs["doc_mask"]).astype(bool)
    if not (qm.all() and dm.all()):
        return _numpy_fallback(inputs)

    from concourse.bass_utils import run_bass_kernel_spmd

    key = ("prog", NDOC)
    if key not in _CACHE:
        _CACHE[key] = _legalize_single_wait(_build_program(NDOC))
    nc = _CACHE[key]

    in_maps = _host_prep(inputs, NDOC, NCORES)
    res = run_bass_kernel_spmd(nc, in_maps, list(range(NCORES)))
    out = np.empty((N,), np.float32)
    for c in range(NCORES):
        grid = res.results[c]["out"]            # [4, NG] -> doc = 4*g + b
        out[c * NDOC:(c + 1) * NDOC] = grid.T.reshape(-1)
    return out


if __name__ == "__main__":
    # quick CoreSim correctness check on a reduced doc count
    from concourse.bass_interp import CoreSim

    nd = int(os.environ.get("SIM_NDOC", "32"))
    rng = np.random.default_rng(0)

    def l2n(x):
        return x / np.sqrt((x * x).sum(-1, keepdims=True) + 1e-12)

    inputs = {
        "query_embs": l2n(rng.standard_normal((NQ, D))).astype(np.float32),
        "doc_embs": l2n(rng.standard_normal((nd, ND, D))).astype(np.float32),
        "importance_weights": rng.random(NQ).astype(np.float32),
        "query_mask": np.ones(NQ, bool),
        "doc_mask": np.ones((nd, ND), bool),
        "asc_w1": (rng.standard_normal((4, 32)) * 0.1).astype(np.float32),
        "asc_b1": np.zeros(32, np.float32),
        "asc_w2": (rng.standard_normal(32) * 0.1).astype(np.float32),
        "asc_b2": np.float32(0.0),
        "asc_blend": np.float32(0.5),
        "mgs_logits": (rng.standard_normal(3) * 0.1).astype(np.float32),
        "tir_w1": (rng.standard_normal((NQ, 64)) * 0.1).astype(np.float32),
        "tir_b1": np.zeros(64, np.float32),
        "tir_w2": (rng.standard_normal(64) * 0.1).astype(np.float32),
        "tir_b2": np.float32(0.0),
    }
    globals()["N"] = nd  # shrink problem for sim
    expected = _numpy_fallback(inputs)

    nc = _build_program(nd)
    in_maps = _host_prep(inputs, nd, 1)
    sim = CoreSim(nc)
    for k, v in in_maps[0].items():
        sim.tensor(k)[:] = v
    sim.simulate()
    grid = np.array(sim.tensor("out"))
    got = grid.T.reshape(-1)
    err = np.abs(got - expected)
    rel = err.max() / np.abs(expected).max()
    print("expected[:8]:", expected[:8])
    print("got[:8]     :", got[:8])
    print("max abs err:", err.max(), " rel:", rel)
